# revision 132
# baseline (speedup 1.0000x reference)
"""Trainium2 Bass kernel for windowed (sparse) attention with memory KV.

Sequence-sharded across 8 NeuronCores: core c computes output tokens
[c*512, (c+1)*512) for both batches and all heads, with a 1-window (128
token) k/v halo. The full attn_bias is never shipped: only the block-
diagonal and sub-diagonal 128x128 blocks each core needs, pre-transposed
and pre-EXPONENTIATED on host (exp(bias), mask folded in as 0 rows) so
the device applies bias multiplicatively AFTER the exp — one Activation
per score block instead of an add+exp pair.

All matmuls run in bf16 (1 PE cycle/row at any output width, vs fp32r's
4x penalty below 256), halving DMA/SBUF for weights and activations.

Device dataflow (per core, per batch):
  kT = Wk.T @ xT                  [1024, 640]   (bf16)
  qT = (Wq*s).T @ xT + bq*s       [1024, 512]   (bf16, ACT bias-add)
  v  = xT.T @ Wv                  [640, 1024]   (token-major, +ones col/head)
  per head pair (row-packed K=64 bf16 matmuls):
    ps[128, 2qw] = [kT_lo.T @ qT_lo | kT_hi.T @ qT_hi]   (both heads packed)
    es = Exp(ps)  (one ACT op)  ->  exp = es * exp_bias  (DVE bf16 2x mode)
    out/sumexp fused: psum[128q, 65] = exp_mem.T@mv_ext + exp_prev.T@vprev
                                       + exp_cur.T@vcur
    out = psum[:, :64] * recip(psum[:, 64])     (per-partition scalar)
  out_all [128q, 1024] -> PE-transpose (bf16) -> y = outT.T @ Wo -> DMA out
"""

import numpy as np

B, N, DIM = 2, 4096, 768
H, DH = 16, 64
W = 128
DI = H * DH                 # 1024
NCORES = 8
TOK = N // NCORES           # 512
NWIN = TOK // W             # 4
KTOK = TOK + W              # 640
NKC = KTOK // W             # 5
KC6 = DIM // 128            # 6 contraction chunks over DIM
DC8 = DI // 128             # 8 chunks over DI

# per-key-chunk q ranges: edge chunks only feed one window -> 128 wide
QW = [W, 2 * W, 2 * W, 2 * W, W]
QLO = [0, 0, W, 2 * W, 3 * W]

USE_DMA_T = True     # XBAR DMA transposes for the output path
USE_WARM = True      # PE clock-ramp keep-alive matmuls
USE_BIG_DMA = True   # consolidated multi-chunk input DMAs
PHASES = "ABCD"


def build_bass():
    import concourse.mybir as mybir
    import concourse.tile as tile
    from concourse import bacc
    from concourse.masks import make_identity
    from contextlib import ExitStack

    f32 = mybir.dt.float32
    bf16 = mybir.dt.bfloat16
    u16 = mybir.dt.uint16
    Exp = mybir.ActivationFunctionType.Exp
    Identity = mybir.ActivationFunctionType.Identity

    nc = bacc.Bacc("TRN2")

    # xkvT: feature-major x with halo, [B*768, 640] (bf16 as u16)
    xkvT_d = nc.dram_tensor("xkvT", [B * DIM, KTOK], u16, kind="ExternalInput")
    # exp(bias), transposed per key-chunk, mask folded as 0, duplicated for
    # both heads of a pair so one DVE mult covers a whole score tile
    ebias_d = nc.dram_tensor("ebias", [B * NKC * W, 4 * W], u16,
                             kind="ExternalInput")
    wq_d = nc.dram_tensor("wq", [DIM, DI], u16, kind="ExternalInput")
    bqs_d = nc.dram_tensor("bqs", [DC8, 128], f32, kind="ExternalInput")
    wkv_d = nc.dram_tensor("wkv", [DIM, 2 * DI], u16, kind="ExternalInput")
    wo_d = nc.dram_tensor("wo", [DI, DIM], u16, kind="ExternalInput")
    memk_d = nc.dram_tensor("memk", [128, 8 * 32], u16, kind="ExternalInput")
    memv_d = nc.dram_tensor("memv", [4, 16 * 65], u16, kind="ExternalInput")
    y_d = nc.dram_tensor("y", [B * TOK, DIM], f32, kind="ExternalOutput")

    with ExitStack() as ctx:
        tc = ctx.enter_context(tile.TileContext(nc))
        # SBUF pools
        const_p = ctx.enter_context(tc.tile_pool(name="const", bufs=1))
        w_p = ctx.enter_context(tc.tile_pool(name="w", bufs=3 * KC6))
        wo_p = ctx.enter_context(tc.tile_pool(name="wo", bufs=DC8))
        xt_p = ctx.enter_context(tc.tile_pool(name="xt", bufs=2))
        oa_p = ctx.enter_context(tc.tile_pool(name="oa", bufs=2))
        kt_p = ctx.enter_context(tc.tile_pool(name="kt", bufs=DC8 + 2))
        qt_p = ctx.enter_context(tc.tile_pool(name="qt", bufs=DC8 + 2))
        v_p = ctx.enter_context(tc.tile_pool(name="v", bufs=2 * NKC))
        es_p = ctx.enter_context(tc.tile_pool(name="es", bufs=4))
        exp_p = ctx.enter_context(tc.tile_pool(name="exp", bufs=32))
        em_p = ctx.enter_context(tc.tile_pool(name="em", bufs=16))
        ot_p = ctx.enter_context(tc.tile_pool(name="ot", bufs=2))
        y_p = ctx.enter_context(tc.tile_pool(name="y", bufs=2))
        rc_p = ctx.enter_context(tc.tile_pool(name="rc", bufs=4))
        # single unified PSUM pool: 8 banks cycling
        ps_p = ctx.enter_context(tc.tile_pool(name="ps", bufs=8, space="PSUM"))

        def pstile(shape, dt=f32):
            pad = 512 if dt == f32 else 1024
            return ps_p.tile(shape, dt, tag="ps", name="ps",
                             padded_shape=[128, pad])

        ident = const_p.tile([128, 128], f32)
        make_identity(nc, ident)
        identb = const_p.tile([128, 128], bf16)
        nc.vector.tensor_copy(identb, ident)

        def warm_pe(n):
            # keep the tensor engine busy (and its clock ramp hot) while
            # DMAs stream in / drain; results are never read
            if not USE_WARM:
                return
            wrm = pstile([128, 512])
            for _ in range(n):
                nc.tensor.matmul(wrm[:, 0:128], identb, identb,
                                 start=True, stop=True)

        # const tiles allocated now, DMAs issued later (off the startup path)
        ebias_sb = const_p.tile([W, B * NKC * 4 * W], bf16)
        # memk padded to 32 cols per head-pair (28 zero cols): the mem-sim
        # matmuls then write full 32-partition psum blocks, so one Exp can
        # read rows 0:64 without touching stale psum bytes
        memk_sb = const_p.tile([128, 8 * 32], bf16)
        # memv duplicated at rows 0:4 and 32:36 so both emem halves (psum
        # rows 0:4 / 32:36) have a base-partition-matched rhs
        memv_sb = const_p.tile([36, 16 * 65], bf16)
        bqs_sb = const_p.tile([128, DC8], f32)
        wo_sb = [wo_p.tile([128, DIM], bf16, tag="wo", name=f"wo{_}")
                 for _ in range(DC8)]

        def load_consts():
            if USE_BIG_DMA:
                nc.gpsimd.dma_start(
                    ebias_sb.rearrange("p (blk c) -> p blk c", c=4 * W),
                    ebias_d.bitcast(bf16).rearrange("(blk p) c -> p blk c",
                                                    p=W))
            else:
                for blk in range(B * NKC):
                    nc.gpsimd.dma_start(
                        ebias_sb[:, blk * 4 * W:(blk + 1) * 4 * W],
                        ebias_d[blk * W:(blk + 1) * W, :].bitcast(bf16))

        def load_wo():
            for d in range(DC8):
                nc.gpsimd.dma_start(
                    wo_sb[d], wo_d[d * 128:(d + 1) * 128, :].bitcast(bf16))

        # startup queue plan (phase A consumes kT -> qT -> v in that order);
        # each weight is ONE consolidated DMA (HWDGE per-op overhead is
        # 625ns, and kT needs all 6 contraction chunks before it can finish
        # a single psum anyway):
        #   sync:   xT(b0), then wq     (kT needs xT first, qT needs wq next)
        #   scalar: wk                  (kT's other operand, concurrent)
        #   gpsimd: bqs+memk+memv, wv, ebias
        nc.gpsimd.dma_start(bqs_sb, bqs_d.rearrange("c p -> p c"))
        nc.gpsimd.dma_start(memk_sb, memk_d[:, :].bitcast(bf16))
        nc.gpsimd.dma_start(memv_sb[0:4], memv_d[:, :].bitcast(bf16))
        nc.gpsimd.dma_start(memv_sb[32:36], memv_d[:, :].bitcast(bf16))
        wq_sb = w_p.tile([128, KC6 * DI], bf16, tag="w", name="wq_sb", bufs=3)
        wqs = [wq_sb[:, k * DI:(k + 1) * DI] for k in range(KC6)]
        wk_sb = w_p.tile([128, KC6 * DI], bf16, tag="w", name="wk_sb", bufs=3)
        wk = [wk_sb[:, k * DI:(k + 1) * DI] for k in range(KC6)]
        # wk in two halves: kT for d8 0-3 only needs the low 512 cols of
        # each chunk, so the first psum can start ~2.4us sooner
        wv_sb = w_p.tile([128, KC6 * DI], bf16, tag="w", name="wv_sb", bufs=3)
        wv = [wv_sb[:, k * DI:(k + 1) * DI] for k in range(KC6)]
        if USE_BIG_DMA:
            for h in range(2):
                nc.scalar.dma_start(
                    wk_sb.rearrange("p (k c) -> p k c", c=DI)[:, :, h * 512:
                                                              (h + 1) * 512],
                    wkv_d[:, :DI].bitcast(bf16)
                    .rearrange("(k p) c -> p k c", p=128)[:, :, h * 512:
                                                          (h + 1) * 512])
            nc.gpsimd.dma_start(
                wv_sb.rearrange("p (k c) -> p k c", c=DI),
                wkv_d[:, DI:].bitcast(bf16).rearrange("(k p) c -> p k c",
                                                      p=128))
        else:
            for d in range(KC6):
                nc.scalar.dma_start(
                    wk[d], wkv_d[d * 128:(d + 1) * 128, :DI].bitcast(bf16))
            for d in range(KC6):
                nc.gpsimd.dma_start(
                    wv[d], wkv_d[d * 128:(d + 1) * 128, DI:].bitcast(bf16))

        # per-batch live state
        S = [dict() for _ in range(B)]

        def emit_xT(b):
            xT_sb = xt_p.tile([128, KC6 * KTOK], bf16, tag="xt", name="xt")
            if USE_BIG_DMA:
                nc.sync.dma_start(
                    xT_sb.rearrange("p (k t) -> p k t", t=KTOK),
                    xkvT_d[b * DIM:(b + 1) * DIM, :].bitcast(bf16)
                    .rearrange("(k p) t -> p k t", p=128))
            else:
                for k in range(KC6):
                    r0 = b * DIM + k * 128
                    nc.sync.dma_start(
                        xT_sb[:, k * KTOK:(k + 1) * KTOK],
                        xkvT_d[r0:r0 + 128, :].bitcast(bf16))
            if b == 0:
                # wq behind xT on the sync queue: lands right when the
                # qT matmuls (after kT) need it
                if USE_BIG_DMA:
                    nc.sync.dma_start(
                        wq_sb.rearrange("p (k c) -> p k c", c=DI),
                        wq_d.bitcast(bf16).rearrange("(k p) c -> p k c",
                                                     p=128))
                else:
                    for d in range(KC6):
                        nc.sync.dma_start(
                            wqs[d],
                            wq_d[d * 128:(d + 1) * 128, :].bitcast(bf16))
            S[b]["xT"] = [xT_sb[:, k * KTOK:(k + 1) * KTOK]
                          for k in range(KC6)]
            S[b]["kT"] = [None] * DC8
            S[b]["qT"] = [None] * DC8
            S[b]["v"] = [None] * NKC
            S[b]["em"] = {}
            S[b]["exp"] = {}

        def emit_kT(b, d8):
            xT = S[b]["xT"]
            kt = kt_p.tile([128, KTOK], bf16, tag="kt", name="kt")
            for nt in range(2):
                ps = pstile([128, 320])
                for k6 in range(KC6):
                    nc.tensor.matmul(
                        ps, wk[k6][:, d8 * 128:(d8 + 1) * 128],
                        xT[k6][:, nt * 320:(nt + 1) * 320],
                        start=(k6 == 0), stop=(k6 == KC6 - 1))
                nc.vector.tensor_copy(kt[:, nt * 320:(nt + 1) * 320], ps)
            S[b]["kT"][d8] = kt

        def emit_qT(b, d8):
            xT = S[b]["xT"]
            qt = qt_p.tile([128, TOK], bf16, tag="qt", name="qt")
            ps = pstile([128, 512])
            for k6 in range(KC6):
                nc.tensor.matmul(
                    ps, wqs[k6][:, d8 * 128:(d8 + 1) * 128],
                    xT[k6][:, W:W + TOK],
                    start=(k6 == 0), stop=(k6 == KC6 - 1))
            nc.scalar.activation(qt, ps, Identity, bias=bqs_sb[:, d8:d8 + 1])
            S[b]["qT"][d8] = qt

        def emit_v(b, tt):
            xT = S[b]["xT"]
            vt = v_p.tile([128, 16 * 65], bf16, tag="v", name="v")
            v3 = vt.rearrange("p (h c) -> p h c", c=65)
            nc.vector.memset(v3[:, :, 64:65], 1.0)
            for half in range(2):
                ps = pstile([128, 512])
                for k6 in range(KC6):
                    nc.tensor.matmul(
                        ps, xT[k6][:, tt * 128:(tt + 1) * 128],
                        wv[k6][:, half * 512:(half + 1) * 512],
                        start=(k6 == 0), stop=(k6 == KC6 - 1))
                nc.vector.tensor_copy(
                    v3[:, half * 8:(half + 1) * 8, 0:64],
                    ps.rearrange("p (h c) -> p h c", c=64))
            S[b]["v"][tt] = vt

        def emit_memsim(b, hp):
            # one bank + one Exp per half, everything at base partition 0
            # (base-32 operands are suspect on hardware)
            qT = S[b]["qT"][hp]
            ems = []
            for h01 in range(2):
                rows = slice(64 * h01, 64 * h01 + 64)
                psm = pstile([128, 512])
                nc.tensor.matmul(
                    psm[0:32], memk_sb[rows, hp * 32:(hp + 1) * 32],
                    qT[rows, :], start=True, stop=True)
                et = em_p.tile([32, 512], bf16, tag="em", name="em")
                nc.scalar.activation(et, psm[0:32], Exp)
                ems.append(et[0:4])
            S[b]["em"][hp] = ems

        def emit_kcsims(b, hp):
            qT, kT = S[b]["qT"][hp], S[b]["kT"][hp]
            exp_tiles = {}
            # one psum bank per sim matmul (matmul writes at non-zero bank
            # column offsets fault on HW); Exp per half, then the exp(bias)
            # mult on DVE (bf16 2x mode)
            for kc in range(NKC):
                qw, qlo = QW[kc], QLO[kc]
                bcol = (b * NKC + kc) * 4 * W
                ex = exp_p.tile([128, 512], bf16, tag="expf",
                                name="expf")[:, :2 * qw]
                for h01 in range(2):
                    rows = slice(64 * h01, 64 * h01 + 64)
                    ps = pstile([128, 512])[:, :qw]
                    nc.tensor.matmul(
                        ps, kT[rows, kc * W:(kc + 1) * W],
                        qT[rows, qlo:qlo + qw], start=True, stop=True)
                    es = es_p.tile([128, 256], bf16, tag="es",
                                   name="es")[:, :qw]
                    nc.scalar.activation(es, ps, Exp)
                    nc.vector.tensor_mul(
                        ex[:, h01 * qw:(h01 + 1) * qw], es,
                        ebias_sb[:, bcol:bcol + qw])
                exp_tiles[kc] = ex
            S[b]["exp"][hp] = exp_tiles

        def emit_av(b, hp, mid=None):
            emem, exp_tiles = S[b]["em"][hp], S[b]["exp"][hp]
            v_ext, out_all = S[b]["v"], S[b]["oa"]
            allgroups = [(w, h01) for w in range(NWIN) for h01 in range(2)]
            for gi in range(0, 8, 4):
                if gi and mid:
                    mid()
                groups = allgroups[gi:gi + 4]
                psvs = {}
                for w, h01 in groups:
                    hg = 2 * hp + h01
                    psvs[(w, h01)] = pstile([128, 65])
                    nc.tensor.matmul(
                        psvs[(w, h01)], emem[h01][:, w * W:(w + 1) * W],
                        memv_sb[0:4, hg * 65:(hg + 1) * 65],
                        start=True, stop=False)
                for w, h01 in groups:
                    hg = 2 * hp + h01
                    pcol = h01 * QW[w] + w * W - QLO[w]
                    nc.tensor.matmul(
                        psvs[(w, h01)], exp_tiles[w][:, pcol:pcol + W],
                        v_ext[w].rearrange("p (h c) -> p h c", c=65)[:, hg],
                        start=False, stop=False)
                for w, h01 in groups:
                    hg = 2 * hp + h01
                    ccol = h01 * QW[w + 1] + w * W - QLO[w + 1]
                    nc.tensor.matmul(
                        psvs[(w, h01)],
                        exp_tiles[w + 1][:, ccol:ccol + W],
                        v_ext[w + 1].rearrange("p (h c) -> p h c", c=65)[:, hg],
                        start=False, stop=True)
                for w, h01 in groups:
                    hg = 2 * hp + h01
                    psv = psvs[(w, h01)]
                    rc = rc_p.tile([128, 1], f32, tag="rc", name="rc")
                    nc.vector.reciprocal(rc, psv[:, 64:65])
                    nc.vector.tensor_scalar_mul(
                        out_all[:, w * DI + hg * 64:w * DI + (hg + 1) * 64],
                        psv[:, 0:64], rc)

        def alloc_otT(b):
            S[b]["otT"] = ot_p.tile([128, NWIN * DI], bf16, tag="ot",
                                    name="ot")

        def emit_outT(b, w, nwin=1, eng=None):
            # XBAR DMA transpose: 3D out AP does all the per-chunk 128x128
            # transposes of `nwin` windows in one descriptor set
            if not USE_DMA_T:
                for ww in range(w, w + nwin):
                    emit_outT_pe(b, ww)
                return
            outT = S[b]["otT"][:, w * DI:(w + nwin) * DI]
            (eng or nc.sync).dma_start_transpose(
                outT.rearrange("p (j q) -> p j q", q=128),
                S[b]["oa"][:, w * DI:(w + nwin) * DI])

        def emit_outT_pe(b, w):
            # PE-side transpose of one window (no DMA latency on the
            # critical tail): 8 chunk transposes + DVE copies
            pss = []
            for d8 in range(DC8):
                ps = pstile([128, 128], bf16)
                nc.tensor.transpose(
                    ps, S[b]["oa"][:, w * DI + d8 * 128:
                                   w * DI + (d8 + 1) * 128], identb)
                pss.append(ps)
            for d8 in range(DC8):
                nc.vector.tensor_copy(
                    S[b]["otT"][:, w * DI + d8 * 128:w * DI + (d8 + 1) * 128],
                    pss[d8])

        def emit_yproj(b, w):
            outT = S[b]["otT"]
            ysb = y_p.tile([128, DIM], f32, tag="y", name="y")
            for nn in range(2):
                ps = pstile([128, 384])
                for d8 in range(DC8):
                    nc.tensor.matmul(
                        ps, outT[:, w * DI + d8 * 128:w * DI + (d8 + 1) * 128],
                        wo_sb[d8][:, nn * 384:(nn + 1) * 384],
                        start=(d8 == 0), stop=(d8 == DC8 - 1))
                nc.vector.tensor_copy(ysb[:, nn * 384:(nn + 1) * 384], ps)
            nc.scalar.dma_start(
                y_d[b * TOK + w * W:b * TOK + (w + 1) * W, :], ysb)

        # ---- phase A: load + full projection of batch 0 ----
        emit_xT(0)
        load_consts()
        warm_pe(82)
        for d8 in range(DC8):
            emit_kT(0, d8)
        for d8 in range(DC8):
            emit_qT(0, d8)
            if d8 > 0:   # memsim(d8-1): its qT is ready, no ACT-latency stall
                emit_memsim(0, d8 - 1)
        emit_memsim(0, DC8 - 1)
        # prepay b0's first score blocks in A's tail: ACT is nearly idle
        # during the projections, shortening phase B's exp chain
        for tt in range(NKC):
            emit_v(0, tt)
            emit_kcsims(0, tt)
        emit_xT(1)
        if "B" not in PHASES:
            nc.compile()
            return nc

        # ---- phase B: attention(b0) interleaved with projection(b1) ----
        S[0]["oa"] = oa_p.tile([128, NWIN * DI], bf16, tag="oa", name="oa")
        for hp in range(DC8):
            if hp == 2:
                load_wo()
            if hp >= 5:   # hp 0-4 were prepaid in phase A
                emit_kcsims(0, hp)
            if hp > 0:
                emit_av(0, hp - 1)
            if hp < 6:   # kT(1,6/7) are deferred to phase C, which is
                emit_kT(1, hp)   # exp-latency-bound and PE-starved
            emit_qT(1, hp)
            if hp > 0:   # b1 mem exps prepaid while ACT has slack (lagged
                emit_memsim(1, hp - 1)   # one iter so qT(1) is ready)
            if hp >= 3:
                emit_v(1, hp - 3)
            if hp >= 5:  # prepay b1's first score blocks too: phase C is
                emit_kcsims(1, hp - 5)   # exp-(ACT-)bound, phase B is not
        alloc_otT(0)
        # b0 window transposes fire the moment their last TSP lands,
        # spread across both HWDGE queues
        emit_av(0, 7, mid=lambda: (emit_outT(0, 0, eng=nc.sync),
                                   emit_outT(0, 1, eng=nc.scalar)))
        emit_outT(0, 2, eng=nc.sync)
        emit_outT(0, 3, eng=nc.scalar)
        emit_memsim(1, DC8 - 1)
        if "C" not in PHASES:
            nc.compile()
            return nc

        # ---- phase C: attention(b1) interleaved with output(b0) ----
        S[1]["oa"] = oa_p.tile([128, NWIN * DI], bf16, tag="oa", name="oa")
        for hp in range(DC8):
            if hp < 2:   # b1's last kT chunks: PE filler for this phase
                emit_kT(1, hp + 6)
            if hp >= 3:   # hp 0-2 were prepaid in phase B
                emit_kcsims(1, hp)
            if hp > 0:
                emit_av(1, hp - 1)
            if hp % 2 == 1:
                emit_yproj(0, hp // 2)
        if "D" not in PHASES:
            nc.compile()
            return nc

        # ---- phase D: output of batch 1 (window 0 transposed on the PE
        # so its yproj starts without DMA latency; windows 1-3 ride one
        # XBAR DMA; warm matmuls bridge the drain at full clock) ----
        alloc_otT(1)
        emit_av(1, 7)
        emit_outT(1, 2, nwin=2)
        emit_outT_pe(1, 0)
        emit_outT_pe(1, 1)
        emit_yproj(1, 0)
        emit_yproj(1, 1)
        warm_pe(6)
        emit_yproj(1, 2)
        emit_yproj(1, 3)
    nc.compile()
    return nc


def _bf16(a):
    import ml_dtypes
    return np.asarray(a, np.float32).astype(ml_dtypes.bfloat16).view(np.uint16)


def host_prep(x, mask, attn_bias, Wq, bq, Wkv, Wo, memory_kv):
    s = np.float32(DH ** -0.5)
    wq = _bf16(np.asarray(Wq, np.float32) * s)
    bqs = (np.asarray(bq, np.float32) * s).astype(np.float32).reshape(DC8, 128)
    wkv = _bf16(Wkv)
    wo = _bf16(Wo)
    x = np.asarray(x, np.float32)
    mask = np.asarray(mask).astype(bool)
    attn_bias = np.asarray(attn_bias, np.float32)
    mk = np.asarray(memory_kv[0], np.float32)
    mv = np.asarray(memory_kv[1], np.float32)

    memk = np.zeros((128, 8 * 32), np.float32)
    for hp in range(8):
        memk[0:64, hp * 32:hp * 32 + 4] = mk[2 * hp].T
        memk[64:128, hp * 32:hp * 32 + 4] = mk[2 * hp + 1].T
    memv = np.zeros((4, 16 * 65), np.float32)
    for h in range(H):
        memv[:, h * 65:h * 65 + 64] = mv[h]
        memv[:, h * 65 + 64] = 1.0

    shared = dict(wq=wq, bqs=bqs, wkv=wkv, wo=wo,
                  memk=_bf16(memk), memv=_bf16(memv))
    xT_full = np.ascontiguousarray(x.transpose(0, 2, 1))    # [B, 768, 4096]
    in_maps = []
    for c in range(NCORES):
        q0 = c * TOK
        xkvT = np.zeros((B, DIM, KTOK), np.float32)
        lo = q0 - W
        src_lo = max(lo, 0)
        xkvT[:, :, src_lo - lo:] = xT_full[:, :, src_lo:q0 + TOK]
        # exp(bias) per key-chunk, masked rows -> 0, q cols per QW/QLO,
        # then duplicated so both heads of a pair share one DVE mult
        ebias = np.zeros((B, NKC, W, 2 * W), np.float32)
        for b in range(B):
            for kc in range(NKC):
                gk = c * NWIN + kc - 1
                if gk < 0:
                    continue
                kr = slice(gk * W, (gk + 1) * W)
                qlo = QLO[kc]
                if kc >= 1:   # this key block is the "current" block of w=kc-1
                    qr = slice((c * NWIN + kc - 1) * W, (c * NWIN + kc) * W)
                    col = (kc - 1) * W - qlo
                    ebias[b, kc, :, col:col + W] = np.exp(attn_bias[b, qr, kr].T)
                if kc <= NWIN - 1:  # and the "previous" block of w=kc
                    qr = slice((c * NWIN + kc) * W, (c * NWIN + kc + 1) * W)
                    col = kc * W - qlo
                    ebias[b, kc, :, col:col + W] = np.exp(attn_bias[b, qr, kr].T)
                kmask = mask[b, gk * W:(gk + 1) * W]
                ebias[b, kc, ~kmask, :] = 0.0
        # duplicate per half: [qw | qw] (qw = QW[kc]; 128-wide chunks use
        # only the first half of their 2W slot)
        ebias_dup = np.zeros((B, NKC, W, 4 * W), np.float32)
        for kc in range(NKC):
            qw = QW[kc]
            ebias_dup[:, kc, :, 0:qw] = ebias[:, kc, :, 0:qw]
            ebias_dup[:, kc, :, qw:2 * qw] = ebias[:, kc, :, 0:qw]
        in_maps.append(dict(
            xkvT=_bf16(xkvT.reshape(B * DIM, KTOK)),
            ebias=_bf16(ebias_dup.reshape(B * NKC * W, 4 * W)),
            **shared))
    return in_maps


_CACHE = {}


def kernel(**inputs):
    import sys
    if "/opt/trn_rl_repo" not in sys.path:
        sys.path.insert(0, "/opt/trn_rl_repo")
    from concourse.bass_utils import run_bass_kernel_spmd

    in_maps = host_prep(**inputs)
    if "nc" not in _CACHE:
        _CACHE["nc"] = build_bass()
    nc = _CACHE["nc"]
    res = run_bass_kernel_spmd(nc, in_maps, core_ids=list(range(NCORES)))
    ys = [res.results[c]["y"].reshape(B, TOK, DIM) for c in range(NCORES)]
    return np.concatenate(ys, axis=1)


if __name__ == "__main__":
    import sys
    sys.path.insert(0, "/opt/trn_rl_repo")
    nc = build_bass()
    print("build OK")
